# revision 1
# baseline (speedup 1.0000x reference)
"""CometAttention Trainium2 kernel.

Computes, for query [B, D] and values [B, S, D] (B=32, S=2048, D=1024, f32):
    w[b, s]   = (query[b] . values[b, s]) / sqrt(D)
    w         = softmax(w, axis=0)            # over the batch dim!
    out[b,s,:] = values[b,s,:] * w[b,s]

Sharding: S is split across 8 NeuronCores (softmax over B is local to each
(s) column, so an S-shard needs no collectives). Each core gets
values[:, c*256:(c+1)*256, :] plus the full query, and produces the matching
output shard; the host concatenates shards along S.

Per-core layout: s-positions are processed 32 at a time in a [128, 8, 1024]
SBUF tile. Partition block si (32 partitions, one per batch) holds the 8
contiguous s-positions s0+8*si .. s0+8*si+7 on the free dim, with d
innermost (32KB contiguous DMA runs); tile[si*32+b, j, :] = values[b,
s0+8*si+j, :]. The batch-dim softmax denominator is computed with one
TensorE matmul against a block-diagonal ones matrix, which both group-sums
over b and broadcasts the result back to all 32 partitions of each group.

Schedule notes (which the TimelineSim cost model confirms matter):
- loads go through the SP HWDGE ring; stores go through the Pool-engine
  SWDGE path. All HWDGE DMAs (SP + ScalarE) round-robin one shared set of
  8 DMAHW lane semaphores, so a store parked on its lane credit (waiting
  for the softmax chain) would transitively stall a LOAD 8 DMAs later on
  the same lane, and a parked store on the ScalarE sequencer head-blocks
  ScalarE compute. Pool stores use the separate DMASW lanes and the
  otherwise-idle Pool sequencer, which removes every mid-kernel DMA gap;
- the query is read from HBM once (128 KiB) and replicated to the other
  three 32-partition groups with cross-partition engine copies, keeping
  3x128 KiB off the DMA engines;
- the weight is applied in place and the vtile doubles as the store source;
- tensor_tensor_reduce is avoided (it faults on this hardware/runtime);
  the dot-product reduction runs as ScalarE Copy-with-accumulate instead.

TimelineSim: 190,392 ns/core = 1,966 lead-in (entry barrier + first DMA
issue latency) + 186,732 DMA busy with zero internal gaps (64 MiB
values+out at the model's 360 B/ns + one 364 ns query read) + 1,694 tail
(DMA completion sem-prop + exit barriers). The DMA-busy portion is the
hard traffic floor, so further gains would have to come out of the ~3.7 us
of fixed framework overhead. A sweep over unit size (jj in {4, 8, 16}),
taper, wave split, v_bufs and chunk_dma confirms every gap-free config
lands within +/-40 ns of this (pure event-packing noise in the model);
jj=4 with a (3, 1) taper is the best found, and its 4-chunk units also
give the finest store granularity (most pipeline slack on real hardware).
"""

import os

import numpy as np
from contextlib import ExitStack

# Defensive: recover NeuronCores left wedged by a previous crashed run.
os.environ.setdefault("NEURON_RT_RESET_CORES", "1")

B = 32
S = 2048
D = 1024
N_CORES = 8
S_SH = S // N_CORES        # 256 s-positions per core
SG = 128 // B              # 4 s-positions per 128-partition group
JJ = 8                     # chunks per DMA unit
S_UNIT = SG * JJ           # 16 s-positions per unit
N_UNITS = S_SH // S_UNIT   # 16 units per core

_CACHE: dict = {}


def _build_nc(jj: int = 4, v_bufs: int = 4, prod_bufs: int = 4,
              wave: int | None = None, taper: tuple = (3, 1),
              chunk_dma: bool = False, pool_stores: bool = True,
              q_replicate: bool = True):
    import concourse.bacc as bacc
    import concourse.mybir as mybir
    import concourse.tile as tile

    f32 = mybir.dt.float32
    Act = mybir.ActivationFunctionType

    nc = bacc.Bacc(
        "TRN2",
        target_bir_lowering=False,
        debug=False,
        enable_asserts=False,
        num_devices=N_CORES,
    )
    values = nc.dram_tensor("values", [B, S_SH, D], f32, kind="ExternalInput")
    query = nc.dram_tensor("query", [B, D], f32, kind="ExternalInput")
    out = nc.dram_tensor("out", [B, S_SH, D], f32, kind="ExternalOutput")
    v_ap, q_ap, o_ap = values.ap(), query.ap(), out.ap()

    with tile.TileContext(nc) as tc, ExitStack() as ctx:
        singles = ctx.enter_context(tc.tile_pool(name="singles", bufs=1))
        vpool = ctx.enter_context(tc.tile_pool(name="vpool", bufs=v_bufs))
        prodpool = ctx.enter_context(tc.tile_pool(name="prodpool", bufs=prod_bufs))
        wpool = ctx.enter_context(tc.tile_pool(name="wpool", bufs=6))
        pspool = ctx.enter_context(tc.tile_pool(name="pspool", bufs=4, space="PSUM"))

        # qtile[si*32 + b, :] = query[b, :]  (loaded via the ScalarE HWDGE
        # ring, which is otherwise idle at startup, so the first values
        # loads on the SP ring begin at t=0)
        qtile = singles.tile([128, D], f32)
        if q_replicate:
            # one 128KiB HBM read, then replicate across the three other
            # partition groups with engine copies (cross-partition-offset
            # copies keep 3*128KiB off the DMA engines)
            nc.scalar.dma_start(out=qtile[0:B, :], in_=q_ap)
            nc.scalar.activation(qtile[B : 2 * B, :], qtile[0:B, :], Act.Copy)
            nc.vector.tensor_copy(qtile[2 * B : 3 * B, :], qtile[0:B, :])
            nc.scalar.activation(qtile[3 * B : 4 * B, :], qtile[0:B, :], Act.Copy)
        else:
            for si in range(SG):
                nc.scalar.dma_start(out=qtile[si * B : (si + 1) * B, :], in_=q_ap)

        # Block-diagonal ones matrix: A[k, m] = 1 iff k//32 == m//32.
        # matmul(out, A, e) then computes out[p, j] = sum_{b in group(p)} e[b, j],
        # i.e. the group sum broadcast back to every partition of the group.
        atile = singles.tile([128, 128], f32)
        nc.vector.memset(atile, 0.0)
        for g in range(SG):
            nc.vector.memset(atile[g * B : (g + 1) * B, g * B : (g + 1) * B], 1.0)

        inv_sqrt_d = 1.0 / float(np.sqrt(D))

        def do_wave(vtile, s0, jj, j_lo, j_hi, osplit=None):
            """Weights + scale + store for chunk range [j_lo, j_hi) of a
            loaded vtile covering s-positions s0 .. s0+SG*jj-1."""
            nw = j_hi - j_lo
            # dot products: wraw[p, j] = sum_d v[p, j, d] * q[b(p), d]/sqrt(D)
            # (DVE elementwise product, then ScalarE copy-with-accumulate for
            # the free-dim reduction; tensor_tensor_reduce faults on this HW)
            wraw = wpool.tile([128, nw], f32, tag="wraw")
            for j in range(j_lo, j_hi):
                prod = prodpool.tile([128, D], f32, tag="prod")
                nc.vector.tensor_mul(prod, vtile[:, j, :], qtile)
                nc.scalar.activation(
                    prod,
                    prod,
                    Act.Copy,
                    scale=inv_sqrt_d,
                    accum_out=wraw[:, j - j_lo : j - j_lo + 1],
                )

            # softmax over b (within each group of 32 partitions)
            etile = wpool.tile([128, nw], f32, tag="etile")
            nc.scalar.activation(etile, wraw, Act.Exp)
            den = pspool.tile([128, nw], f32, tag="den")
            nc.tensor.matmul(den, atile, etile, start=True, stop=True)
            rec = wpool.tile([128, nw], f32, tag="rec")
            nc.vector.reciprocal(rec, den)
            wfin = wpool.tile([128, nw], f32, tag="wfin")
            nc.vector.tensor_mul(wfin, etile, rec)

            # scale values by the per-(b, s) weight, in place (vtile's last
            # reader is the dot-product mul, which already ran), and store.
            # tensor_scalar on DVE runs at 2x for f32 SBUF; ScalarE takes the
            # other half to balance engine occupancy. Stores are issued via
            # the Pool SWDGE path (separate DMASW lanes + idle sequencer) so
            # a parked store can neither lane-couple to SP-ring loads nor
            # head-block ScalarE compute.
            for j in range(j_lo, j_hi):
                if j % 2 == 0:
                    nc.vector.tensor_scalar_mul(
                        vtile[:, j, :], vtile[:, j, :],
                        wfin[:, j - j_lo : j - j_lo + 1]
                    )
                else:
                    nc.scalar.activation(
                        vtile[:, j, :],
                        vtile[:, j, :],
                        Act.Copy,
                        scale=wfin[:, j - j_lo : j - j_lo + 1],
                    )
            store_eng = nc.gpsimd if pool_stores else nc.scalar
            if osplit is not None:
                for j in range(j_lo, j_hi):
                    store_eng.dma_start(
                        out=osplit[:, :, j, :].transpose([1, 0, 2]),
                        in_=vtile[:, j, :],
                    )
            else:
                for si in range(SG):
                    store_eng.dma_start(
                        out=o_ap[:, s0 + jj * si + j_lo : s0 + jj * si + j_hi, :],
                        in_=vtile[si * B : (si + 1) * B, j_lo:j_hi, :],
                    )

        # unit sizes: uniform jj chunks, except optional tapered tail units
        # (smaller final units shorten the post-last-load compute tail)
        sizes = []
        s_total = S_SH // SG  # total chunks per core
        tail = sum(taper)
        assert (s_total - tail) % jj == 0
        sizes = [jj] * ((s_total - tail) // jj) + list(taper)

        w = wave
        s0 = 0
        for ujj in sizes:
            vtile = vpool.tile([128, jj, D], f32, tag="vtile")
            if chunk_dma:
                # one full-width [128, 1024] DMA per chunk: partition block si
                # holds s = s0 + si*ujj + j, so the source AP for chunk j is
                # [si(stride ujj*D), b(stride S_SH*D), d] — 3 dims. Compute on
                # chunk j can start as soon as its own 512KB lands.
                vsplit = v_ap[:, s0 : s0 + SG * ujj, :].rearrange(
                    "b (si j) d -> b si j d", si=SG, j=ujj
                )
                osplit = o_ap[:, s0 : s0 + SG * ujj, :].rearrange(
                    "b (si j) d -> b si j d", si=SG, j=ujj
                )
                for j in range(ujj):
                    nc.sync.dma_start(
                        out=vtile[:, j, :],
                        in_=vsplit[:, :, j, :].transpose([1, 0, 2]),
                    )
            else:
                for si in range(SG):
                    nc.sync.dma_start(
                        out=vtile[si * B : (si + 1) * B, 0:ujj, :],
                        in_=v_ap[:, s0 + ujj * si : s0 + ujj * si + ujj, :],
                    )
            uw = w or ujj
            for j_lo in range(0, ujj, uw):
                do_wave(vtile, s0, ujj, j_lo, min(j_lo + uw, ujj),
                        osplit if chunk_dma else None)
            s0 += SG * ujj

    nc.compile()
    return nc


def _get_nc():
    if "nc" not in _CACHE:
        _CACHE["nc"] = _build_nc()
    return _CACHE["nc"]


def kernel(query: np.ndarray, values: np.ndarray) -> np.ndarray:
    from concourse import bass_utils

    nc = _get_nc()
    query = np.ascontiguousarray(np.asarray(query, dtype=np.float32))
    values = np.asarray(values, dtype=np.float32)
    in_maps = [
        {
            "values": np.ascontiguousarray(values[:, c * S_SH : (c + 1) * S_SH, :]),
            "query": query,
        }
        for c in range(N_CORES)
    ]
    last_exc = None
    for attempt, backoff in enumerate((20.0, 30.0, 45.0, 60.0, 90.0)):
        try:
            res = bass_utils.run_bass_kernel_spmd(
                nc, in_maps, core_ids=list(range(N_CORES))
            )
            return np.concatenate([r["out"] for r in res.results], axis=1)
        except ModuleNotFoundError as e:
            # BASS_TRACE=1 requests NTFF profiling, whose axon hook module is
            # not shipped in every container; fall back to an untraced run.
            os.environ["BASS_NEVER_TRACE"] = "1"
            last_exc = e
            continue
        except Exception as e:
            # A crashed previous run can leave a NeuronCore transiently
            # wedged (NRT_EXEC_UNIT_UNRECOVERABLE); NEURON_RT_RESET_CORES=1
            # recovers it on a fresh NRT session. Best effort: drop the jax
            # backend so the retry reconnects, and give the wedged core
            # escalating time to clear (observed: a wedge that outlasts
            # quick retries can still clear within a minute or two).
            last_exc = e
            import time as _time

            try:
                import jax.extend as _jex

                _jex.backend.clear_backends()
            except Exception:
                pass
            _time.sleep(backoff)
    raise last_exc



# revision 16
# speedup vs baseline: 1.9532x; 1.9532x over previous
"""CometAttention Trainium2 kernel (bf16 I/O).

Computes, for query [B, D] and values [B, S, D] (B=32, S=2048, D=1024, f32):
    w[b, s]   = (query[b] . values[b, s]) / sqrt(D)
    w         = softmax(w, axis=0)            # over the batch dim!
    out[b,s,:] = values[b,s,:] * w[b,s]

Sharding: S is split across 8 NeuronCores (softmax over B is local to each
(s) column, so an S-shard needs no collectives). Each core gets
values[:, c*256:(c+1)*256, :] plus the full query, and produces the matching
output shard; the host concatenates shards along S.

Traffic: values are shipped to the device as bfloat16 and the output shard is
returned as bfloat16 (converted back to f32 on the host). That halves HBM
traffic vs f32 — 16 MiB in + 16 MiB out per core — and the max elementwise
error this introduces (~0.4% from value quantization plus ~1% worst-case from
the quantized values entering the logits) sits well inside the 2e-2 gate.
The query stays f32 and the dot product accumulates in f32, so the score
path adds no further error.

Per-core layout: s-positions are processed 32 at a time in a [128, 8, 1024]
SBUF tile. Partition block si (32 partitions, one per batch) holds the 8
contiguous s-positions s0+8*si .. s0+8*si+7 on the free dim, with d
innermost (16KB contiguous DMA runs); tile[si*32+b, j, :] = values[b,
s0+8*si+j, :]. The batch-dim softmax denominator is computed with one
TensorE matmul against a block-diagonal ones matrix, which both group-sums
over b and broadcasts the result back to all 32 partitions of each group.

Engine budget per [128, 1024] chunk (1456 ns of DMA: 728 load + 728 store):
- DVE: affine_mul_reduce fuses the q*v product with the free-dim reduction
  in one 1127 ns pass (tensor_tensor_reduce faults on this runtime; the
  equivalent custom-DVE op executes correctly), plus reciprocal/weight smalls.
- Act: the output scale (Copy-with-per-partition-scale, 1038 ns) and exp.
- PE: the block-diagonal group-sum matmul (tens of ns).
- Pool: SWDGE stores (separate DMASW lanes; keeps the shared HWDGE device
  and the SP load ring free of store descriptor generation).
All engines sit at <=80% of the DMA pace, so the single DMA_ENGINES device
(360 B/ns aggregate) stays gap-free: ~93.6 us of traffic + ~2 us lead-in
+ tail.
"""

import os

import numpy as np
from contextlib import ExitStack

# Defensive: recover NeuronCores left wedged by a previous crashed run.
os.environ.setdefault("NEURON_RT_RESET_CORES", "1")

B = 32
S = 2048
D = 1024
N_CORES = 8
S_SH = S // N_CORES        # 256 s-positions per core
SG = 128 // B              # 4 partition groups (s-positions per chunk)
JJ = 8                     # chunks per DMA unit

_CACHE: dict = {}


def _build_nc(jj: int = JJ, v_bufs: int = 8, prod_bufs: int = 2,
              taper: tuple = (6, 2), split_dma: bool = False,
              wfin_pool: bool = False, w_bufs: int = 6,
              store_splits: int = 4):
    import concourse.bacc as bacc
    import concourse.mybir as mybir
    import concourse.tile as tile

    f32 = mybir.dt.float32
    bf16 = mybir.dt.bfloat16
    Act = mybir.ActivationFunctionType

    nc = bacc.Bacc(
        "TRN2",
        target_bir_lowering=False,
        debug=False,
        enable_asserts=False,
        num_devices=N_CORES,
    )
    values = nc.dram_tensor("values", [B, S_SH, D], bf16, kind="ExternalInput")
    query = nc.dram_tensor("query", [B, D], f32, kind="ExternalInput")
    out = nc.dram_tensor("out", [B, S_SH, D], bf16, kind="ExternalOutput")
    v_ap, q_ap, o_ap = values.ap(), query.ap(), out.ap()

    inv_sqrt_d = 1.0 / float(np.sqrt(D))

    with tile.TileContext(nc) as tc, ExitStack() as ctx:
        singles = ctx.enter_context(tc.tile_pool(name="singles", bufs=1))
        vpool = ctx.enter_context(tc.tile_pool(name="vpool", bufs=v_bufs))
        prodpool = ctx.enter_context(tc.tile_pool(name="prodpool", bufs=prod_bufs))
        wpool = ctx.enter_context(tc.tile_pool(name="wpool", bufs=w_bufs))
        pspool = ctx.enter_context(tc.tile_pool(name="pspool", bufs=4, space="PSUM"))

        # qtile[si*32 + b, :] = query[b, :] (f32): one DMA with a stride-0
        # leading dim reads the 128KiB query from HBM four times, filling all
        # four partition groups with no engine copies on the critical path.
        # Emitted first on the SP ring so it precedes the first values load
        # on the DMA device and qtile is ready before the first chunk lands.
        qtile = singles.tile([128, D], f32)
        nc.sync.dma_start(
            out=qtile[0 : 2 * B, :],
            in_=q_ap.rearrange("b (o d) -> o b d", o=1).broadcast_to([2, B, D]),
        )
        nc.vector.tensor_copy(qtile[2 * B : 4 * B, :], qtile[0 : 2 * B, :])

        # Block-diagonal ones matrix: A[k, m] = 1 iff k//32 == m//32.
        # matmul(out, A, e) computes out[p, j] = sum_{b in group(p)} e[b, j],
        # i.e. the group sum broadcast back to every partition of the group.
        atile = singles.tile([128, 128], f32)
        nc.vector.memset(atile, 0.0)
        for g in range(SG):
            nc.vector.memset(atile[g * B : (g + 1) * B, g * B : (g + 1) * B], 1.0)

        # --- software-pipelined unit emission -------------------------------
        # Per steady-state iteration k the emission order is
        #   load(k), AMR(k,0..1), recip(k-1), wfin(k-1), AMR(k,2..),
        #   exp(k), mm(k), scales(k-1), store(k-1)
        # so on Act the exp(k) lands BEFORE the scales of unit k-1: the
        # exp->matmul->recip->wfin chain for unit k overlaps the 8.3 us of
        # unit k-1 scales instead of serializing into Act's loop (which
        # would add ~1.9 us of Act idle per unit: measured 10.37 us Act
        # cycle vs 8.5 us busy). Likewise recip/wfin sit two AMRs into the
        # next unit so they never head-block the in-order DVE queue.
        def rearr(ap):
            return ap.rearrange("b (si j) d -> b si (j d)", si=SG).transpose(
                [1, 0, 2]
            )

        def emit_load(s0, ujj, load_slices=1):
            # One DMA covers the whole unit: HBM s-index s0+ujj*si+j means
            # the four si partition groups own four CONTIGUOUS s-ranges, so
            # the HBM AP is [si (stride ujj*D), b (stride S_SH*D), j*d] — 3
            # dims, si-major to match the si-major partition order. One DMA
            # instruction per unit keeps the SWDGE/HWDGE descriptor
            # generators (994/625 ns fixed cost per DMA) off the critical
            # path.
            vtile = vpool.tile([128, ujj, D], bf16, tag="vtile")
            if load_slices > 1:
                # j-column slices so the first AMRs can start after ~1/nth
                # of the unit has landed (used for the first unit to cut the
                # pipeline fill time).
                bounds = [ujj * t // load_slices for t in range(load_slices + 1)]
                for t in range(load_slices):
                    j_lo, j_hi = bounds[t], bounds[t + 1]
                    if j_lo == j_hi:
                        continue
                    vsub = v_ap[:, s0 : s0 + SG * ujj, :].rearrange(
                        "b (si j) d -> b si j d", si=SG
                    )[:, :, j_lo:j_hi, :].rearrange("b si j d -> b si (j d)")
                    nc.sync.dma_start(
                        out=vtile[:, j_lo:j_hi, :], in_=vsub.transpose([1, 0, 2])
                    )
            else:
                nc.sync.dma_start(
                    out=vtile, in_=rearr(v_ap[:, s0 : s0 + SG * ujj, :])
                )
            return vtile

        def emit_amrs(vtile, ujj, j_lo, j_hi):
            # wraw[p, j] = sum_d (v[p, j, d]/sqrt(D)) * q[b(p), d], f32
            # accum, one fused DVE pass per chunk.
            for j in range(j_lo, j_hi):
                prod = prodpool.tile([128, D], f32, tag="prod")
                nc.vector.affine_mul_reduce(
                    out=prod,
                    accum_out=state["wraw"][:, j : j + 1],
                    in0=vtile[:, j, :],
                    in1=qtile,
                    scale=inv_sqrt_d,
                    bias=0.0,
                )

        def emit_exp_mm(ujj):
            # softmax over b (within each group of 32 partitions): exp, then
            # one PE matmul against the block-diagonal ones matrix for the
            # group-sum-and-broadcast denominator.
            etile = wpool.tile([128, ujj], f32, tag="etile")
            nc.scalar.activation(etile, state["wraw"], Act.Exp)
            den = pspool.tile([128, ujj], f32, tag="den")
            nc.tensor.matmul(den, atile, etile, start=True, stop=True)
            return etile, den

        def emit_recip_wfin(etile, den, ujj):
            rec = wpool.tile([128, ujj], f32, tag="rec")
            nc.vector.reciprocal(rec, den)
            wfin = wpool.tile([128, ujj], f32, tag="wfin")
            nc.vector.tensor_mul(wfin, etile, rec)
            return wfin

        def emit_scales_store(vtile, wfin, s0, ujj):
            # scale values by the per-(b, s) weight in place (vtile's last
            # reader was the fused dot product) and store via Pool SWDGE.
            # Stores go out in `store_splits` j-column slices so a slice
            # becomes DMA-ready as soon as its scales ran — the back half of
            # the run is drained by stores alone, and finer slices keep the
            # DMA device from idling between whole-unit completions. The
            # sliced HBM AP is [si (stride ujj*D), b, j-range*d] with the
            # same si-major partition order as the load.
            nsp = max(1, min(store_splits, ujj))
            bounds = [ujj * t // nsp for t in range(nsp + 1)]
            for t in range(nsp):
                for j in range(bounds[t], bounds[t + 1]):
                    nc.scalar.activation(
                        vtile[:, j, :],
                        vtile[:, j, :],
                        Act.Copy,
                        scale=wfin[:, j : j + 1],
                    )
                j_lo, j_hi = bounds[t], bounds[t + 1]
                osub = o_ap[:, s0 : s0 + SG * ujj, :].rearrange(
                    "b (si j) d -> b si j d", si=SG
                )[:, :, j_lo:j_hi, :].rearrange("b si j d -> b si (j d)")
                nc.gpsimd.dma_start(
                    out=osub.transpose([1, 0, 2]),
                    in_=vtile[:, j_lo:j_hi, :],
                )

        # unit sizes: uniform jj chunks, except tapered tail units (smaller
        # final units shorten the post-last-load compute tail)
        s_total = S_SH // SG  # total chunks per core
        tail = sum(taper)
        assert (s_total - tail) % jj == 0
        sizes = [jj] * ((s_total - tail) // jj) + [t for t in taper if t]
        starts = [SG * sum(sizes[:i]) for i in range(len(sizes))]

        state: dict = {}
        prev = None  # (vtile, etile, den, s0, ujj) of unit k-1
        for k, (s0, ujj) in enumerate(zip(starts, sizes)):
            vtile = emit_load(s0, ujj, load_slices=(4 if k == 0 else 1))
            wraw = wpool.tile([128, ujj], f32, tag="wraw")
            state["wraw"] = wraw
            emit_amrs(vtile, ujj, 0, min(2, ujj))
            if prev is not None:
                pv, pe, pd, ps0, pujj = prev
                wfin = emit_recip_wfin(pe, pd, pujj)
            emit_amrs(vtile, ujj, min(2, ujj), ujj)
            etile, den = emit_exp_mm(ujj)
            if prev is not None:
                emit_scales_store(pv, wfin, ps0, pujj)
            prev = (vtile, etile, den, s0, ujj)
        pv, pe, pd, ps0, pujj = prev
        wfin = emit_recip_wfin(pe, pd, pujj)
        emit_scales_store(pv, wfin, ps0, pujj)

    nc.compile()
    return nc


def _get_nc():
    if "nc" not in _CACHE:
        _CACHE["nc"] = _build_nc()
    return _CACHE["nc"]


def kernel(query: np.ndarray, values: np.ndarray) -> np.ndarray:
    import ml_dtypes
    from concourse import bass_utils

    nc = _get_nc()
    bf16 = ml_dtypes.bfloat16
    query = np.ascontiguousarray(np.asarray(query, dtype=np.float32))
    values = np.asarray(values, dtype=np.float32)
    in_maps = [
        {
            "values": np.ascontiguousarray(
                values[:, c * S_SH : (c + 1) * S_SH, :].astype(bf16)
            ),
            "query": query,
        }
        for c in range(N_CORES)
    ]
    last_exc = None
    for attempt, backoff in enumerate((20.0, 30.0, 45.0, 60.0, 90.0)):
        try:
            res = bass_utils.run_bass_kernel_spmd(
                nc, in_maps, core_ids=list(range(N_CORES))
            )
            return np.concatenate(
                [r["out"].astype(np.float32) for r in res.results], axis=1
            )
        except ModuleNotFoundError as e:
            # BASS_TRACE=1 requests NTFF profiling, whose axon hook module is
            # not shipped in every container; fall back to an untraced run.
            os.environ["BASS_NEVER_TRACE"] = "1"
            last_exc = e
            continue
        except Exception as e:
            # A crashed previous run can leave a NeuronCore transiently
            # wedged (NRT_EXEC_UNIT_UNRECOVERABLE); NEURON_RT_RESET_CORES=1
            # recovers it on a fresh NRT session. Best effort: drop the jax
            # backend so the retry reconnects, and give the wedged core
            # escalating time to clear.
            last_exc = e
            import time as _time

            try:
                import jax.extend as _jex

                _jex.backend.clear_backends()
            except Exception:
                pass
            _time.sleep(backoff)
    raise last_exc


# revision 19
# speedup vs baseline: 1.9547x; 1.0008x over previous
"""CometAttention Trainium2 kernel (bf16 I/O, fused dot-product reduce).

Computes, for query [B, D] and values [B, S, D] (B=32, S=2048, D=1024, f32):
    w[b, s]   = (query[b] . values[b, s]) / sqrt(D)
    w         = softmax(w, axis=0)            # over the batch dim!
    out[b,s,:] = values[b,s,:] * w[b,s]

Sharding: S is split across 8 NeuronCores (the batch-dim softmax is local to
each s column, so an S-shard needs no collectives). Each core gets
values[:, c*256:(c+1)*256, :] plus the full query and produces the matching
output shard; the host concatenates shards along S.

Traffic: values are shipped to the device as bfloat16 and the output shard
returns as bfloat16 (converted back to f32 on the host). That halves HBM
traffic vs f32 — 16 MiB in + 16 MiB out per core, 93.2 us at the model's
360 B/ns DMA bandwidth (loads and stores serialize on the single DMA-engine
pool). The query stays f32 and the dot product accumulates in f32, so the
quantization error is ~0.4% from the values plus ~1% worst-case from the
quantized values entering the logits: measured max elementwise rel err
1.41e-2 against the f32 reference, inside the 2e-2 gate with margin.

Per-core layout: 32 s-positions per [128, 8, 1024] SBUF unit. Partition
p = si*32 + b holds s-positions s0+8*si .. s0+8*si+7 on the free dim with d
innermost, so one 3-dim DMA AP [si (stride 8D), b (stride S_SH*D), (j d)]
covers a whole unit: 16 KiB contiguous runs, one DMA instruction per unit
load (the four si s-ranges are contiguous in s). Stores go out in four
2-chunk slices of the same shape so a slice becomes DMA-ready as soon as its
two scales ran.

Engine assignment per [128, 1024] chunk (the DMA pace is 1456 ns/chunk:
728 load + 728 store):
- DVE: affine_mul_reduce fuses the (v/sqrt(D))*q product with the free-dim
  reduction into one 1127 ns pass, accumulating f32 into wraw[:, j]
  (tensor_tensor_reduce, the native fused op, faults on this runtime; the
  custom-DVE op executes correctly and was validated against numpy).
  Plus the per-unit reciprocal and the wfin = e * (1/den) multiply.
- Act: the output scale (Copy with per-partition f32 scale, 1038 ns) and
  the per-unit Exp.
- PE: one matmul per unit against a block-diagonal ones matrix [128, 128]
  (A[k, m] = 1 iff k//32 == m//32), which group-sums exp over b and
  broadcasts the softmax denominator back to all 32 partitions per group.
- Pool: SWDGE store descriptor generation (994 ns fixed per DMA), off the
  shared HWDGE device that loads use.

Schedule: the emission is software-pipelined — per iteration k:
  load(k), AMR(k,0..1), recip(k-1), wfin(k-1), AMR(k,2..7),
  exp(k), den-matmul(k), scales(k-1) + sliced stores(k-1)
so exp(k) lands on Act BEFORE the scales of unit k-1: the
exp->matmul->recip->wfin chain of each unit overlaps the previous unit's
8.3 us of scales instead of serializing into Act's loop (without this, Act
paces the kernel at 10.4 us/unit and the DMA idles ~20%). recip/wfin sit
two AMRs into the next unit so they never head-block the in-order DVE
queue. All 8+2 unit buffers are SBUF-resident (v_bufs=8, ~141 KB of the
192 KB per partition), the first unit's load is sliced so compute starts
after the first 1/4 lands, the query is read once and replicated with two
DVE copies, and a (6, 2) taper shortens the post-last-load tail.

TimelineSim: 97,401 ns/core = 1,966 lead-in + 93,935 DMA busy (93,207
values+out, 364 query, one 286 ns gap) + 1,594 exit (DMA-completion
sem-prop + exit barriers), vs the 190,392 ns f32 baseline. The DMA-busy
portion is the bf16 traffic floor; the elementwise 2e-2 gate rules out
8-bit output, so further gains would have to come out of ~3.6 us of fixed
framework overhead.
"""

import os

import numpy as np
from contextlib import ExitStack

# Defensive: recover NeuronCores left wedged by a previous crashed run.
os.environ.setdefault("NEURON_RT_RESET_CORES", "1")

B = 32
S = 2048
D = 1024
N_CORES = 8
S_SH = S // N_CORES        # 256 s-positions per core
SG = 128 // B              # 4 partition groups (s-positions per chunk)
JJ = 8                     # chunks per DMA unit

_CACHE: dict = {}


def _build_nc(jj: int = JJ, v_bufs: int = 8, prod_bufs: int = 2,
              taper: tuple = (6, 2), split_dma: bool = False,
              wfin_pool: bool = False, w_bufs: int = 6,
              store_splits: int = 4, first_slices: int = 4,
              second_slices: int = 1):
    import concourse.bacc as bacc
    import concourse.mybir as mybir
    import concourse.tile as tile

    f32 = mybir.dt.float32
    bf16 = mybir.dt.bfloat16
    Act = mybir.ActivationFunctionType

    nc = bacc.Bacc(
        "TRN2",
        target_bir_lowering=False,
        debug=False,
        enable_asserts=False,
        num_devices=N_CORES,
    )
    values = nc.dram_tensor("values", [B, S_SH, D], bf16, kind="ExternalInput")
    query = nc.dram_tensor("query", [B, D], f32, kind="ExternalInput")
    out = nc.dram_tensor("out", [B, S_SH, D], bf16, kind="ExternalOutput")
    v_ap, q_ap, o_ap = values.ap(), query.ap(), out.ap()

    inv_sqrt_d = 1.0 / float(np.sqrt(D))

    with tile.TileContext(nc) as tc, ExitStack() as ctx:
        singles = ctx.enter_context(tc.tile_pool(name="singles", bufs=1))
        vpool = ctx.enter_context(tc.tile_pool(name="vpool", bufs=v_bufs))
        prodpool = ctx.enter_context(tc.tile_pool(name="prodpool", bufs=prod_bufs))
        wpool = ctx.enter_context(tc.tile_pool(name="wpool", bufs=w_bufs))
        pspool = ctx.enter_context(tc.tile_pool(name="pspool", bufs=4, space="PSUM"))

        # qtile[si*32 + b, :] = query[b, :] (f32): one DMA with a stride-0
        # leading dim reads the 128KiB query from HBM four times, filling all
        # four partition groups with no engine copies on the critical path.
        # Emitted first on the SP ring so it precedes the first values load
        # on the DMA device and qtile is ready before the first chunk lands.
        qtile = singles.tile([128, D], f32)
        nc.sync.dma_start(out=qtile[0:B, :], in_=q_ap)
        nc.vector.tensor_copy(qtile[B : 2 * B, :], qtile[0:B, :])
        nc.vector.tensor_copy(qtile[2 * B : 4 * B, :], qtile[0 : 2 * B, :])

        # Block-diagonal ones matrix: A[k, m] = 1 iff k//32 == m//32.
        # matmul(out, A, e) computes out[p, j] = sum_{b in group(p)} e[b, j],
        # i.e. the group sum broadcast back to every partition of the group.
        atile = singles.tile([128, 128], f32)
        nc.vector.memset(atile, 0.0)
        for g in range(SG):
            nc.vector.memset(atile[g * B : (g + 1) * B, g * B : (g + 1) * B], 1.0)

        # --- software-pipelined unit emission -------------------------------
        # Per steady-state iteration k the emission order is
        #   load(k), AMR(k,0..1), recip(k-1), wfin(k-1), AMR(k,2..),
        #   exp(k), mm(k), scales(k-1), store(k-1)
        # so on Act the exp(k) lands BEFORE the scales of unit k-1: the
        # exp->matmul->recip->wfin chain for unit k overlaps the 8.3 us of
        # unit k-1 scales instead of serializing into Act's loop (which
        # would add ~1.9 us of Act idle per unit: measured 10.37 us Act
        # cycle vs 8.5 us busy). Likewise recip/wfin sit two AMRs into the
        # next unit so they never head-block the in-order DVE queue.
        def rearr(ap):
            return ap.rearrange("b (si j) d -> b si (j d)", si=SG).transpose(
                [1, 0, 2]
            )

        def emit_load(s0, ujj, load_slices=1):
            # One DMA covers the whole unit: HBM s-index s0+ujj*si+j means
            # the four si partition groups own four CONTIGUOUS s-ranges, so
            # the HBM AP is [si (stride ujj*D), b (stride S_SH*D), j*d] — 3
            # dims, si-major to match the si-major partition order. One DMA
            # instruction per unit keeps the SWDGE/HWDGE descriptor
            # generators (994/625 ns fixed cost per DMA) off the critical
            # path.
            vtile = vpool.tile([128, ujj, D], bf16, tag="vtile")
            if load_slices > 1:
                # j-column slices so the first AMRs can start after ~1/nth
                # of the unit has landed (used for the first unit to cut the
                # pipeline fill time).
                bounds = [ujj * t // load_slices for t in range(load_slices + 1)]
                for t in range(load_slices):
                    j_lo, j_hi = bounds[t], bounds[t + 1]
                    if j_lo == j_hi:
                        continue
                    vsub = v_ap[:, s0 : s0 + SG * ujj, :].rearrange(
                        "b (si j) d -> b si j d", si=SG
                    )[:, :, j_lo:j_hi, :].rearrange("b si j d -> b si (j d)")
                    nc.sync.dma_start(
                        out=vtile[:, j_lo:j_hi, :], in_=vsub.transpose([1, 0, 2])
                    )
            else:
                nc.sync.dma_start(
                    out=vtile, in_=rearr(v_ap[:, s0 : s0 + SG * ujj, :])
                )
            return vtile

        def emit_amrs(vtile, ujj, j_lo, j_hi):
            # wraw[p, j] = sum_d (v[p, j, d]/sqrt(D)) * q[b(p), d], f32
            # accum, one fused DVE pass per chunk.
            for j in range(j_lo, j_hi):
                prod = prodpool.tile([128, D], f32, tag="prod")
                nc.vector.affine_mul_reduce(
                    out=prod,
                    accum_out=state["wraw"][:, j : j + 1],
                    in0=vtile[:, j, :],
                    in1=qtile,
                    scale=inv_sqrt_d,
                    bias=0.0,
                )

        def emit_exp_mm(ujj):
            # softmax over b (within each group of 32 partitions): exp, then
            # one PE matmul against the block-diagonal ones matrix for the
            # group-sum-and-broadcast denominator.
            etile = wpool.tile([128, ujj], f32, tag="etile")
            nc.scalar.activation(etile, state["wraw"], Act.Exp)
            den = pspool.tile([128, ujj], f32, tag="den")
            nc.tensor.matmul(den, atile, etile, start=True, stop=True)
            return etile, den

        def emit_recip_wfin(etile, den, ujj):
            rec = wpool.tile([128, ujj], f32, tag="rec")
            nc.vector.reciprocal(rec, den)
            wfin = wpool.tile([128, ujj], f32, tag="wfin")
            nc.vector.tensor_mul(wfin, etile, rec)
            return wfin

        def emit_scales_store(vtile, wfin, s0, ujj):
            # scale values by the per-(b, s) weight in place (vtile's last
            # reader was the fused dot product) and store via Pool SWDGE.
            # Stores go out in `store_splits` j-column slices so a slice
            # becomes DMA-ready as soon as its scales ran — the back half of
            # the run is drained by stores alone, and finer slices keep the
            # DMA device from idling between whole-unit completions. The
            # sliced HBM AP is [si (stride ujj*D), b, j-range*d] with the
            # same si-major partition order as the load.
            nsp = max(1, min(store_splits, ujj))
            bounds = [ujj * t // nsp for t in range(nsp + 1)]
            for t in range(nsp):
                for j in range(bounds[t], bounds[t + 1]):
                    nc.scalar.activation(
                        vtile[:, j, :],
                        vtile[:, j, :],
                        Act.Copy,
                        scale=wfin[:, j : j + 1],
                    )
                j_lo, j_hi = bounds[t], bounds[t + 1]
                osub = o_ap[:, s0 : s0 + SG * ujj, :].rearrange(
                    "b (si j) d -> b si j d", si=SG
                )[:, :, j_lo:j_hi, :].rearrange("b si j d -> b si (j d)")
                nc.gpsimd.dma_start(
                    out=osub.transpose([1, 0, 2]),
                    in_=vtile[:, j_lo:j_hi, :],
                )

        # unit sizes: uniform jj chunks, except tapered tail units (smaller
        # final units shorten the post-last-load compute tail)
        s_total = S_SH // SG  # total chunks per core
        tail = sum(taper)
        assert (s_total - tail) % jj == 0
        sizes = [jj] * ((s_total - tail) // jj) + [t for t in taper if t]
        starts = [SG * sum(sizes[:i]) for i in range(len(sizes))]

        state: dict = {}
        prev = None  # (vtile, etile, den, s0, ujj) of unit k-1
        for k, (s0, ujj) in enumerate(zip(starts, sizes)):
            vtile = emit_load(s0, ujj, load_slices=(first_slices if k == 0 else (second_slices if k == 1 else 1)))
            wraw = wpool.tile([128, ujj], f32, tag="wraw")
            state["wraw"] = wraw
            emit_amrs(vtile, ujj, 0, min(2, ujj))
            if prev is not None:
                pv, pe, pd, ps0, pujj = prev
                wfin = emit_recip_wfin(pe, pd, pujj)
            emit_amrs(vtile, ujj, min(2, ujj), ujj)
            etile, den = emit_exp_mm(ujj)
            if prev is not None:
                emit_scales_store(pv, wfin, ps0, pujj)
            prev = (vtile, etile, den, s0, ujj)
        pv, pe, pd, ps0, pujj = prev
        wfin = emit_recip_wfin(pe, pd, pujj)
        emit_scales_store(pv, wfin, ps0, pujj)

    nc.compile()
    return nc


def _get_nc():
    if "nc" not in _CACHE:
        _CACHE["nc"] = _build_nc()
    return _CACHE["nc"]


def kernel(query: np.ndarray, values: np.ndarray) -> np.ndarray:
    import ml_dtypes
    from concourse import bass_utils

    nc = _get_nc()
    bf16 = ml_dtypes.bfloat16
    query = np.ascontiguousarray(np.asarray(query, dtype=np.float32))
    values = np.asarray(values, dtype=np.float32)
    in_maps = [
        {
            "values": np.ascontiguousarray(
                values[:, c * S_SH : (c + 1) * S_SH, :].astype(bf16)
            ),
            "query": query,
        }
        for c in range(N_CORES)
    ]
    last_exc = None
    for attempt, backoff in enumerate((20.0, 30.0, 45.0, 60.0, 90.0)):
        try:
            res = bass_utils.run_bass_kernel_spmd(
                nc, in_maps, core_ids=list(range(N_CORES))
            )
            return np.concatenate(
                [r["out"].astype(np.float32) for r in res.results], axis=1
            )
        except ModuleNotFoundError as e:
            # BASS_TRACE=1 requests NTFF profiling, whose axon hook module is
            # not shipped in every container; fall back to an untraced run.
            os.environ["BASS_NEVER_TRACE"] = "1"
            last_exc = e
            continue
        except Exception as e:
            # A crashed previous run can leave a NeuronCore transiently
            # wedged (NRT_EXEC_UNIT_UNRECOVERABLE); NEURON_RT_RESET_CORES=1
            # recovers it on a fresh NRT session. Best effort: drop the jax
            # backend so the retry reconnects, and give the wedged core
            # escalating time to clear.
            last_exc = e
            import time as _time

            try:
                import jax.extend as _jex

                _jex.backend.clear_backends()
            except Exception:
                pass
            _time.sleep(backoff)
    raise last_exc
